# revision 1
# baseline (speedup 1.0000x reference)
"""CrossTrackAttention Trainium2 kernel (8-core SPMD, batch x head-group sharding).

Reference computation (B=2, S=2048, D=1024, H=16, HD=64):
    qkv = x @ w_qkv + b_qkv
    q, k, v per head; scores = q k^T / sqrt(HD); masked softmax with a
    [B, S, S] bool mask; out = (attn @ v) @ w_out + b_out.

Sharding: core c handles batch c//4 and heads [4*(c%4), 4*(c%4)+4).  The
[B,H,S,S] score tensor partitions cleanly along B and H, so there are no
cross-device comms; the per-core partial outputs (each over 4 heads' feature
rows of w_out) are summed on the host.

Device algorithm per core (transpose-free flash attention):
  - host passes x^T, so QKV projections produce q^T/k^T in [feature, token]
    layout directly (lhsT = w block, rhs = x^T block) and v in natural
    [token, feature] layout (lhsT = x^T block, rhs = w block).
  - scores are computed transposed, s^T[k, q] (lhsT = k^T slice, rhs = q^T
    slice), so softmax numerator exp(s - 20) runs on ACT over wide-q tiles
    and the attention@V matmul consumes p^T tiles as lhsT with no transposes.
  - a ones column appended to V accumulates the softmax denominator in the
    same PSUM accumulation group; out = p~ @ [v|1] then row-scaled by the
    reciprocal of the denominator (softmax is shift invariant, so the fixed
    -20 shift cancels).
  - 1/sqrt(HD) is folded into w_q/b_q on the host; b_v is folded into the
    output bias (sum_k softmax = 1  =>  +b_v passes through attention), so
    the device never touches b_v.

Two compiled variants:
  - "structured": the cross-track mask of the reference's setup_inputs()
    (causal within each of 2 tracks of 1024 tokens + bidirectional same-bar
    cross-track attention, BAR=64).  Block-sparse schedule with a constant
    128x128 triangular tile for the causal diagonal; no mask DMA at all.
  - "generic": any other [B, S, S] bool mask; dense scores multiplied by the
    0/1 mask (streamed as bf16).
"""

import numpy as np
import ml_dtypes

import concourse.bass as bass
import concourse.mybir as mybir
import concourse.tile as tile
from concourse import bacc
from concourse.bass_utils import run_bass_kernel_spmd
from concourse.masks import make_identity, make_upper_triangular

B, S, D, H = 2, 2048, 1024, 16
HD = D // H
N_TRACKS = 2
BAR = 64
TL = S // N_TRACKS            # 1024 tokens per track
N_CORES = 8
HPC = H // (N_CORES // B)     # 4 heads per core
FPC = HPC * HD                # 256 features per core
DT = mybir.dt
BF16 = ml_dtypes.bfloat16

_cache: dict = {}


def _structured_mask() -> np.ndarray:
    idx = np.arange(S)
    track = idx // TL
    pos = idx % TL
    bar = pos // BAR
    same_track = track[:, None] == track[None, :]
    causal = pos[:, None] >= pos[None, :]
    same_bar = bar[:, None] == bar[None, :]
    return (same_track & causal) | (~same_track & same_bar)


def _build(structured: bool):
    nc = bacc.Bacc()
    f32, bf16 = DT.float32, DT.bfloat16

    xT = nc.declare_dram_parameter("xT", [128, 8, S], bf16, isOutput=False)
    wq = nc.declare_dram_parameter("wq", [128, 8, FPC], bf16, isOutput=False)
    wk = nc.declare_dram_parameter("wk", [128, 8, FPC], bf16, isOutput=False)
    wv = nc.declare_dram_parameter("wv", [128, 8, FPC], bf16, isOutput=False)
    wo = nc.declare_dram_parameter("wo", [128, 2, D], bf16, isOutput=False)
    bq = nc.declare_dram_parameter("bq", [128, 2], f32, isOutput=False)
    bk = nc.declare_dram_parameter("bk", [128, 2], f32, isOutput=False)
    if structured:
        bm_d = nc.declare_dram_parameter("bm", [128, 128], bf16, isOutput=False)
    else:
        maskT = nc.declare_dram_parameter("maskT", [S, S], bf16, isOutput=False)
    out_d = nc.declare_dram_parameter("out", [S, D], f32, isOutput=True)

    with tile.TileContext(nc) as tc:
        with (
            tc.tile_pool(name="consts", bufs=1) as consts,
            tc.tile_pool(name="pp", bufs=36) as ppool,
            tc.tile_pool(name="small", bufs=6) as small,
            tc.tile_pool(name="mp", bufs=4) as mpool,
            tc.tile_pool(name="ps512", bufs=2, space="PSUM") as ps512,
            tc.tile_pool(name="scps", bufs=2, space="PSUM") as scps,
            tc.tile_pool(name="avps", bufs=2, space="PSUM") as avps,
        ):
            Copy = mybir.ActivationFunctionType.Copy
            Exp = mybir.ActivationFunctionType.Exp

            xt_sb = consts.tile([128, 8, S], bf16)
            nc.sync.dma_start(out=xt_sb, in_=xT[:, :, :])
            wq_sb = consts.tile([128, 8, FPC], bf16)
            nc.sync.dma_start(out=wq_sb, in_=wq[:, :, :])
            wk_sb = consts.tile([128, 8, FPC], bf16)
            nc.sync.dma_start(out=wk_sb, in_=wk[:, :, :])
            wv_sb = consts.tile([128, 8, FPC], bf16)
            nc.sync.dma_start(out=wv_sb, in_=wv[:, :, :])
            wo_sb = consts.tile([128, 2, D], bf16)
            nc.sync.dma_start(out=wo_sb, in_=wo[:, :, :])
            bq_sb = consts.tile([128, 2], f32)
            nc.sync.dma_start(out=bq_sb, in_=bq[:, :])
            bk_sb = consts.tile([128, 2], f32)
            nc.sync.dma_start(out=bk_sb, in_=bk[:, :])

            m20 = consts.tile([128, 1], f32)
            nc.vector.memset(m20, -20.0)
            ident = consts.tile([128, 128], bf16)
            make_identity(nc, ident)
            if structured:
                tri = consts.tile([128, 128], bf16)
                make_upper_triangular(nc, tri, val=1.0, diag=True)
                # cross-track 128x128 tile is block-diagonal in 64-token bars
                bm = consts.tile([128, 128], bf16)
                nc.sync.dma_start(out=bm, in_=bm_d[:, :])

            qT_sb = consts.tile([128, 2, S], bf16)
            kT_sb = consts.tile([128, 2, S], bf16)
            # v' tiles: per k-tile, 4 heads x (64 v columns + ones column)
            v_sb = consts.tile([128, 16, HPC * (HD + 1)], bf16)
            v4 = v_sb.rearrange("p k (h c) -> p k h c", c=HD + 1)
            nc.gpsimd.memset(v4[:, :, :, HD : HD + 1], 1.0)
            attn_sb = consts.tile([128, 16, FPC], bf16)
            attnT_sb = consts.tile([128, 2, S], bf16)

            # ---------------- projections ----------------
            for dst, w_sb, b_sb in ((qT_sb, wq_sb, bq_sb), (kT_sb, wk_sb, bk_sb)):
                for ft in range(2):
                    for qb in range(4):
                        ps = ps512.tile([128, 512], f32, tag="ps512")
                        for dt_i in range(8):
                            nc.tensor.matmul(
                                ps,
                                w_sb[:, dt_i, ft * 128 : (ft + 1) * 128],
                                xt_sb[:, dt_i, qb * 512 : (qb + 1) * 512],
                                start=(dt_i == 0),
                                stop=(dt_i == 7),
                            )
                        nc.vector.tensor_scalar_add(
                            out=dst[:, ft, qb * 512 : (qb + 1) * 512],
                            in0=ps,
                            scalar1=b_sb[:, ft : ft + 1],
                        )
            for tb in range(16):
                ps = ps512.tile([128, FPC], f32, tag="ps512")
                for dt_i in range(8):
                    nc.tensor.matmul(
                        ps,
                        xt_sb[:, dt_i, tb * 128 : (tb + 1) * 128],
                        wv_sb[:, dt_i, :],
                        start=(dt_i == 0),
                        stop=(dt_i == 7),
                    )
                nc.any.tensor_copy(
                    out=v4[:, tb, :, 0:HD],
                    in_=ps.rearrange("p (h c) -> p h c", c=HD),
                )

            # ---------------- attention ----------------
            NQC = TL // 128  # 8 q-chunks per track
            for h in range(HPC):
                fth, hh = h // 2, h % 2
                prow = slice(hh * 64, hh * 64 + 64)

                def _av_block(tbg, mms, h=h, fth=fth, prow=prow):
                    av = avps.tile([128, HD + 1], f32, tag="av")
                    for j, (lh, ktg) in enumerate(mms):
                        nc.tensor.matmul(
                            av, lh, v4[:, ktg, h, :],
                            start=(j == 0), stop=(j == len(mms) - 1),
                        )
                    r = small.tile([128, 1], f32, tag="recip")
                    nc.vector.reciprocal(r, av[:, HD : HD + 1])
                    nc.vector.tensor_scalar_mul(
                        attn_sb[:, tbg, h * 64 : (h + 1) * 64], av[:, 0:HD], r
                    )
                    tp = avps.tile([128, 128], bf16, tag="av")
                    nc.tensor.transpose(
                        tp[0:64, :], attn_sb[:, tbg, h * 64 : (h + 1) * 64], ident
                    )
                    nc.vector.tensor_copy(
                        out=attnT_sb[prow, fth, tbg * 128 : (tbg + 1) * 128],
                        in_=tp[0:64, :],
                    )

                ptiles = {}
                if structured:
                    # per k-tile (track t, local i): q columns cover
                    # [own-track q from 128*i to track end | cross-track 128]
                    pcross = {}
                    for t in range(2):
                        for i in range(NQC):
                            wA = TL - 128 * i
                            wT = wA + 128
                            ktg = t * NQC + i
                            lhsT = kT_sb[prow, fth, ktg * 128 : (ktg + 1) * 128]
                            split = wT > 1024
                            scw = wA if split else wT
                            sc = scps.tile([128, 1024], f32, tag="scps")
                            col = 0
                            while col < wA:
                                wseg = min(512, wA - col)
                                qg = t * TL + 128 * i + col
                                nc.tensor.matmul(
                                    sc[:, col : col + wseg],
                                    lhsT,
                                    qT_sb[prow, fth, qg : qg + wseg],
                                    start=True,
                                    stop=True,
                                )
                                col += wseg
                            qg = (1 - t) * TL + 128 * i
                            if split:
                                scx = avps.tile([128, 128], f32, tag="av")
                                nc.tensor.matmul(
                                    scx, lhsT,
                                    qT_sb[prow, fth, qg : qg + 128],
                                    start=True, stop=True,
                                )
                                px = small.tile([128, 128], bf16, tag="ppx")
                                nc.scalar.activation(
                                    out=px, in_=scx, func=Exp, bias=m20, scale=1.0,
                                )
                                nc.vector.tensor_mul(px, px, bm)
                                pcross[(t, i)] = (px, 0)
                            else:
                                nc.tensor.matmul(
                                    sc[:, wA:wT], lhsT,
                                    qT_sb[prow, fth, qg : qg + 128],
                                    start=True, stop=True,
                                )
                            pt = ppool.tile([128, 1024], bf16, tag="pp")
                            nc.scalar.activation(
                                out=pt[:, 0:scw], in_=sc[:, 0:scw], func=Exp,
                                bias=m20, scale=1.0,
                            )
                            nc.vector.tensor_mul(pt[:, 0:128], pt[:, 0:128], tri)
                            if not split:
                                nc.vector.tensor_mul(pt[:, wA:wT], pt[:, wA:wT], bm)
                                pcross[(t, i)] = (pt, wA)
                            ptiles[(t, i)] = pt
                    for t in range(2):
                        for qc in range(NQC):
                            mms = []
                            for i in range(qc + 1):
                                mms.append(
                                    (ptiles[(t, i)][:, 128 * (qc - i) : 128 * (qc - i) + 128],
                                     t * NQC + i)
                                )
                            pxt, xoff = pcross[(1 - t, qc)]
                            mms.append(
                                (pxt[:, xoff : xoff + 128], (1 - t) * NQC + qc)
                            )
                            _av_block(t * NQC + qc, mms)
                else:
                    # dense: per q-half, all k-tiles then the AV for that half
                    for half in range(2):
                        ptiles = {}
                        for ktg in range(16):
                            lhsT = kT_sb[prow, fth, ktg * 128 : (ktg + 1) * 128]
                            sc = scps.tile([128, 1024], f32, tag="scps")
                            for seg in range(2):
                                qg = half * 1024 + seg * 512
                                nc.tensor.matmul(
                                    sc[:, seg * 512 : (seg + 1) * 512],
                                    lhsT,
                                    qT_sb[prow, fth, qg : qg + 512],
                                    start=True,
                                    stop=True,
                                )
                            pt = ppool.tile([128, 1024], bf16, tag="pp")
                            nc.scalar.activation(
                                out=pt[:, 0:1024], in_=sc[:, 0:1024], func=Exp,
                                bias=m20, scale=1.0,
                            )
                            mt = mpool.tile([128, 1024], bf16, tag="mp")
                            nc.sync.dma_start(
                                out=mt,
                                in_=maskT[ktg * 128 : (ktg + 1) * 128,
                                          half * 1024 : (half + 1) * 1024],
                            )
                            nc.vector.tensor_mul(pt[:, 0:1024], pt[:, 0:1024], mt)
                            ptiles[ktg] = pt
                        for qc in range(NQC):
                            mms = [
                                (ptiles[ktg][:, 128 * qc : 128 * qc + 128], ktg)
                                for ktg in range(16)
                            ]
                            _av_block(half * NQC + qc, mms)

            # ---------------- output projection ----------------
            for tb in range(16):
                for ob in range(2):
                    ps = ps512.tile([128, 512], f32, tag="ps512")
                    for ftt in range(2):
                        nc.tensor.matmul(
                            ps,
                            attnT_sb[:, ftt, tb * 128 : (tb + 1) * 128],
                            wo_sb[:, ftt, ob * 512 : (ob + 1) * 512],
                            start=(ftt == 0),
                            stop=(ftt == 1),
                        )
                    ot = small.tile([128, 512], f32, tag="outstage")
                    nc.any.tensor_copy(out=ot, in_=ps)
                    nc.sync.dma_start(
                        out=out_d[tb * 128 : (tb + 1) * 128, ob * 512 : (ob + 1) * 512],
                        in_=ot,
                    )
    nc.finalize()
    return nc


def _get_nc(structured: bool):
    key = "structured" if structured else "generic"
    if key not in _cache:
        _cache[key] = _build(structured)
    return _cache[key]


def kernel(x, cross_track_mask, w_qkv, b_qkv, w_out, b_out):
    x = np.asarray(x, dtype=np.float32)
    mask = np.asarray(cross_track_mask).astype(bool)
    w_qkv = np.asarray(w_qkv, dtype=np.float32)
    b_qkv = np.asarray(b_qkv, dtype=np.float32)
    w_out = np.asarray(w_out, dtype=np.float32)
    b_out = np.asarray(b_out, dtype=np.float32)

    structured = bool(np.array_equal(mask, np.broadcast_to(_structured_mask(), mask.shape)))
    nc = _get_nc(structured)

    scale = 1.0 / np.sqrt(np.float32(HD))
    b_v = b_qkv[2 * D :]
    b_out_adj = (b_out + b_v @ w_out).astype(np.float32)

    in_maps = []
    for c in range(N_CORES):
        b = c // (N_CORES // B)
        g = c % (N_CORES // B)
        fs = slice(g * FPC, (g + 1) * FPC)

        xT_c = np.ascontiguousarray(
            x[b].T.reshape(8, 128, S).transpose(1, 0, 2)
        ).astype(BF16)

        def wslice(off):
            w = w_qkv[:, off + g * FPC : off + (g + 1) * FPC]
            return np.ascontiguousarray(
                w.reshape(8, 128, FPC).transpose(1, 0, 2)
            )

        wq_c = (wslice(0) * scale).astype(BF16)
        wk_c = wslice(D).astype(BF16)
        wv_c = wslice(2 * D).astype(BF16)
        bq_c = np.ascontiguousarray(
            (b_qkv[fs] * scale).reshape(2, 128).T
        ).astype(np.float32)
        bk_c = np.ascontiguousarray(
            b_qkv[D + g * FPC : D + (g + 1) * FPC].reshape(2, 128).T
        ).astype(np.float32)
        wo_c = np.ascontiguousarray(
            w_out[fs].reshape(2, 128, D).transpose(1, 0, 2)
        ).astype(BF16)

        m = {
            "xT": xT_c,
            "wq": wq_c,
            "wk": wk_c,
            "wv": wv_c,
            "wo": wo_c,
            "bq": bq_c,
            "bk": bk_c,
        }
        if structured:
            ar = np.arange(128)
            m["bm"] = ((ar[:, None] // BAR) == (ar[None, :] // BAR)).astype(BF16)
        else:
            m["maskT"] = np.ascontiguousarray(mask[b].T).astype(BF16)
        in_maps.append(m)

    res = run_bass_kernel_spmd(nc, in_maps, list(range(N_CORES)))

    out = np.empty((B, S, D), dtype=np.float32)
    gpb = N_CORES // B
    for b in range(B):
        acc = res.results[b * gpb]["out"].astype(np.float32)
        for g in range(1, gpb):
            acc = acc + res.results[b * gpb + g]["out"]
        out[b] = acc + b_out_adj
    return out



# revision 35
# speedup vs baseline: 1.2599x; 1.2599x over previous
"""CrossTrackAttention Trainium2 kernel (8-core SPMD, batch x head-group sharding).

Reference computation (B=2, S=2048, D=1024, H=16, HD=64):
    qkv = x @ w_qkv + b_qkv
    q, k, v per head; scores = q k^T / sqrt(HD); masked softmax with a
    [B, S, S] bool mask; out = (attn @ v) @ w_out + b_out.

Sharding: core c handles batch c//4 and heads [4*(c%4), 4*(c%4)+4).  The
[B,H,S,S] score tensor partitions cleanly along B and H, so there are no
cross-device comms; the per-core partial outputs (each over 4 heads' feature
rows of w_out) are summed on the host.

Device algorithm per core (transpose-free flash attention):
  - host passes x^T, so QKV projections produce q^T/k^T in [feature, token]
    layout directly (lhsT = w block, rhs = x^T block) and v in natural
    [token, feature] layout (lhsT = x^T block, rhs = w block).
  - bias handling: b_k and the q.b_k cross terms are constant over keys and
    cancel in softmax, so they are dropped.  b_q contributes b_q.k_j per key
    j; since keys are the PARTITION dim of the transposed score tiles, that
    term is a per-partition scalar and is folded into the exp's bias input:
    kappa = x @ (W_k b_q * scale), exp(s + kappa - 20).  No bias adds on the
    vector engine at all; b_v passes through softmax into the output bias
    (host-side).
  - scores are computed transposed, s^T[k, q] (lhsT = k^T slice, rhs = q^T
    slice), so the exp runs over wide-q tiles and the attention@V matmul
    consumes p^T tiles as lhsT with no transposes.
  - a ones column appended to V accumulates the softmax denominator in the
    same PSUM accumulation group; out = p~ @ [v|1] then row-scaled by the
    reciprocal of the denominator (softmax is shift invariant, so the fixed
    -20 shift cancels).
  - PSUM->SBUF staging copies run on GpSimd (Pool); masks on DVE; exp on ACT;
    the final projection streams PSUM directly to DRAM over DMA.
  - emission interleaves projections with per-head attention so the tensor
    engine stays busy while ACT/DVE chew the softmax of the previous head.

Two compiled variants:
  - "structured": the cross-track mask of the reference's setup_inputs()
    (causal within each of 2 tracks of 1024 tokens + bidirectional same-bar
    cross-track attention, BAR=64).  Block-sparse schedule with a constant
    128x128 triangular tile for the causal diagonal; no mask DMA at all.
  - "generic": any other [B, S, S] bool mask; dense scores multiplied by the
    0/1 mask (streamed as bf16).
"""

import numpy as np
import ml_dtypes

import concourse.bass as bass
import concourse.mybir as mybir
import concourse.tile as tile
from concourse import bacc
from concourse.bass_utils import run_bass_kernel_spmd
from concourse.masks import make_identity, make_upper_triangular

B, S, D, H = 2, 2048, 1024, 16
HD = D // H
N_TRACKS = 2
BAR = 64
TL = S // N_TRACKS            # 1024 tokens per track
N_CORES = 8
HPC = H // (N_CORES // B)     # 4 heads per core
FPC = HPC * HD                # 256 features per core
DT = mybir.dt
BF16 = ml_dtypes.bfloat16

_cache: dict = {}


def _structured_mask() -> np.ndarray:
    idx = np.arange(S)
    track = idx // TL
    pos = idx % TL
    bar = pos // BAR
    same_track = track[:, None] == track[None, :]
    causal = pos[:, None] >= pos[None, :]
    same_bar = bar[:, None] == bar[None, :]
    return (same_track & causal) | (~same_track & same_bar)


def _build_structured():
    nc = bacc.Bacc()
    f32, bf16 = DT.float32, DT.bfloat16

    xT = nc.declare_dram_parameter("xT", [128, 8, S], bf16, isOutput=False)
    wq = nc.declare_dram_parameter("wq", [128, 8, FPC], bf16, isOutput=False)
    wk = nc.declare_dram_parameter("wk", [128, 8, FPC], bf16, isOutput=False)
    wv = nc.declare_dram_parameter("wv", [128, 8, FPC], bf16, isOutput=False)
    wo = nc.declare_dram_parameter("wo", [128, 2, D], bf16, isOutput=False)
    kap = nc.declare_dram_parameter("kap", [128, 16, HPC], f32, isOutput=False)
    bm_d = nc.declare_dram_parameter("bm", [128, 128], bf16, isOutput=False)
    out_d = nc.declare_dram_parameter("out", [S, D], bf16, isOutput=True)

    NQC = TL // 128  # 8 q-chunks per track
    # per k-tile index i, the wide p tile holds [own-track q cols | cross q
    # cols] = wA + 128 (except i=0, whose cross block lives in a separate px
    # tile).  All 4 heads' tiles stay resident, so pools are sized per width
    # class: i=0 and i=1 share width 1024, i>=2 use 1152-128*i.
    PW = {i: (1024 if i <= 1 else 1152 - 128 * i) for i in range(NQC)}

    with tile.TileContext(nc) as tc:
        with (
            tc.tile_pool(name="consts", bufs=1) as consts,
            tc.tile_pool(name="pp1024", bufs=16) as pp1024,
            tc.tile_pool(name="pp896", bufs=8) as pp896,
            tc.tile_pool(name="pp768", bufs=8) as pp768,
            tc.tile_pool(name="pp640", bufs=8) as pp640,
            tc.tile_pool(name="pp512", bufs=8) as pp512,
            tc.tile_pool(name="pp384", bufs=8) as pp384,
            tc.tile_pool(name="pp256", bufs=8) as pp256,
            tc.tile_pool(name="pxp", bufs=8) as pxp,
            tc.tile_pool(name="small", bufs=6) as small,
            tc.tile_pool(name="outs", bufs=4) as outs,
            tc.tile_pool(name="ps512", bufs=2, space="PSUM") as ps512,
            tc.tile_pool(name="scps", bufs=2, space="PSUM") as scps,
            tc.tile_pool(name="avps", bufs=2, space="PSUM") as avps,
        ):
            Exp = mybir.ActivationFunctionType.Exp
            ppools = {1024: pp1024, 896: pp896, 768: pp768, 640: pp640,
                      512: pp512, 384: pp384, 256: pp256}

            # ---------------- constant loads ----------------
            kap_sb = consts.tile([128, 16, HPC], f32)
            nc.sync.dma_start(out=kap_sb, in_=kap[:, :, :])
            xt_sb = consts.tile([128, 8, S], bf16)
            nc.sync.dma_start(out=xt_sb[:, :, 0:128], in_=xT[:, :, 0:128])
            wq_sb = consts.tile([128, 8, FPC], bf16)
            nc.sync.dma_start(out=wq_sb, in_=wq[:, :, :])
            wk_sb = consts.tile([128, 8, FPC], bf16)
            nc.sync.dma_start(out=wk_sb, in_=wk[:, :, :])
            nc.sync.dma_start(out=xt_sb[:, :, 128:512], in_=xT[:, :, 128:512])
            for qb in range(1, 4):
                nc.sync.dma_start(
                    out=xt_sb[:, :, qb * 512 : (qb + 1) * 512],
                    in_=xT[:, :, qb * 512 : (qb + 1) * 512],
                )
            wv_sb = consts.tile([128, 8, FPC], bf16)
            nc.sync.dma_start(out=wv_sb, in_=wv[:, :, :])
            wo_sb = consts.tile([128, 2, D], bf16)
            nc.sync.dma_start(out=wo_sb, in_=wo[:, :, :])
            bm = consts.tile([128, 128], bf16)
            nc.sync.dma_start(out=bm, in_=bm_d[:, :])

            ident = consts.tile([128, 128], bf16)
            make_identity(nc, ident)
            tri = consts.tile([128, 128], bf16)
            make_upper_triangular(nc, tri, val=1.0, diag=True)

            qT_sb = consts.tile([128, 2, S], bf16)
            kT_sb = consts.tile([128, 2, S], bf16)
            # v' tiles: per k-tile, 4 heads x (64 v columns + ones column)
            v_sb = consts.tile([128, 16, HPC * (HD + 1)], bf16)
            v4 = v_sb.rearrange("p k (h c) -> p k h c", c=HD + 1)
            nc.gpsimd.memset(v4[:, :, :, HD : HD + 1], 1.0)
            attn_sb = consts.tile([128, 16, FPC], bf16)
            attnT_sb = consts.tile([128, 2, S], bf16)


            # ---------------- emission helpers ----------------
            def emit_q_proj_cols(ft, c0, c1):
                ps = ps512.tile([128, c1 - c0], f32, tag="ps512")
                for dt_i in range(8):
                    nc.tensor.matmul(
                        ps,
                        wq_sb[:, dt_i, ft * 128 : (ft + 1) * 128],
                        xt_sb[:, dt_i, c0:c1],
                        start=(dt_i == 0),
                        stop=(dt_i == 7),
                    )
                nc.vector.tensor_copy(out=qT_sb[:, ft, c0:c1], in_=ps)

            def emit_q_proj(ft, qb):
                emit_q_proj_cols(ft, qb * 512, (qb + 1) * 512)

            def emit_k_proj_cols(ft, c0, c1):
                ps = ps512.tile([128, c1 - c0], f32, tag="ps512")
                for dt_i in range(8):
                    nc.tensor.matmul(
                        ps,
                        wk_sb[:, dt_i, ft * 128 : (ft + 1) * 128],
                        xt_sb[:, dt_i, c0:c1],
                        start=(dt_i == 0),
                        stop=(dt_i == 7),
                    )
                nc.vector.tensor_copy(out=kT_sb[:, ft, c0:c1], in_=ps)

            def emit_k_proj(ft, ktg):
                # one 128-token k-tile so scores can start early
                emit_k_proj_cols(ft, ktg * 128, (ktg + 1) * 128)

            def emit_v_proj(tb):
                ps = ps512.tile([128, FPC], f32, tag="ps512")
                for dt_i in range(8):
                    nc.tensor.matmul(
                        ps,
                        xt_sb[:, dt_i, tb * 128 : (tb + 1) * 128],
                        wv_sb[:, dt_i, :],
                        start=(dt_i == 0),
                        stop=(dt_i == 7),
                    )
                nc.vector.tensor_copy(
                    out=v4[:, tb, :, 0:HD],
                    in_=ps.rearrange("p (h c) -> p h c", c=HD),
                )

            # per-head score state: pt[(h, t, i)] -> wide p tile,
            # px[(h, t, i)] -> (tile, col offset of the 128-wide cross block)
            pt_tiles: dict = {}
            px_tiles: dict = {}

            def emit_score_tile(h, t, i):
                fth, hh = h // 2, h % 2
                prow = slice(hh * 64, hh * 64 + 64)
                wA = TL - 128 * i
                wT = wA + 128
                ktg = t * NQC + i
                lhsT = kT_sb[prow, fth, ktg * 128 : (ktg + 1) * 128]
                kapb = kap_sb[:, ktg, h : h + 1]
                split = wT > 1024
                scw = wA if split else wT
                sc = scps.tile([128, 1024], f32, tag="scps")
                col = 0
                while col < wA:
                    wseg = min(512, wA - col)
                    qg = t * TL + 128 * i + col
                    nc.tensor.matmul(
                        sc[:, col : col + wseg],
                        lhsT,
                        qT_sb[prow, fth, qg : qg + wseg],
                        start=True,
                        stop=True,
                    )
                    col += wseg
                qg = (1 - t) * TL + 128 * i
                if split:
                    scx = avps.tile([128, 128], f32, tag="av")
                    nc.tensor.matmul(
                        scx, lhsT, qT_sb[prow, fth, qg : qg + 128],
                        start=True, stop=True,
                    )
                    px = pxp.tile([128, 128], bf16, tag="ppx")
                    nc.scalar.activation(
                        out=px, in_=scx, func=Exp, bias=kapb, scale=1.0,
                    )
                    nc.gpsimd.tensor_mul(px, px, bm)
                    px_tiles[(h, t, i)] = (px, 0)
                else:
                    nc.tensor.matmul(
                        sc[:, wA:wT], lhsT,
                        qT_sb[prow, fth, qg : qg + 128],
                        start=True, stop=True,
                    )
                pw = PW[i]
                pt = ppools[pw].tile([128, pw], bf16, tag="pp")
                nc.scalar.activation(
                    out=pt[:, 0:scw], in_=sc[:, 0:scw], func=Exp,
                    bias=kapb, scale=1.0,
                )
                nc.vector.tensor_mul(pt[:, 0:128], pt[:, 0:128], tri)
                if not split:
                    nc.gpsimd.tensor_mul(pt[:, wA:wT], pt[:, wA:wT], bm)
                    px_tiles[(h, t, i)] = (pt, wA)
                pt_tiles[(h, t, i)] = pt

            def emit_av_a(h, t, qc, av=None, hslot=0):
                """AV matmuls + denominator reciprocal + row-scale into
                attn_sb (PE then DVE; no PE instruction waits on DVE)."""
                tbg = t * NQC + qc
                mms = []
                for i in range(qc + 1):
                    mms.append(
                        (pt_tiles[(h, t, i)][:, 128 * (qc - i) : 128 * (qc - i) + 128],
                         t * NQC + i)
                    )
                pxt, xoff = px_tiles[(h, 1 - t, qc)]
                mms.append((pxt[:, xoff : xoff + 128], (1 - t) * NQC + qc))

                avv = avps.tile([128, HD + 1], f32, tag="av")
                for j, (lh, ktg) in enumerate(mms):
                    nc.tensor.matmul(
                        avv, lh, v4[:, ktg, h, :],
                        start=(j == 0), stop=(j == len(mms) - 1),
                    )
                r = small.tile([128, 1], f32, tag="recip")
                nc.vector.reciprocal(r, avv[:, HD : HD + 1])
                nc.vector.tensor_scalar_mul(
                    attn_sb[:, tbg, h * 64 : (h + 1) * 64], avv[:, 0:HD], r
                )

            def emit_av_pair_a(h0, h1, t, qc):
                av = avps.tile([128, 2 * (HD + 1)], f32, tag="av")
                emit_av_a(h0, t, qc, av=av, hslot=0)
                emit_av_a(h1, t, qc, av=av, hslot=1)

            def emit_av_b(h, t, qc):
                """Transpose scaled attention into attnT (lagged so the PE
                transpose never waits on the part-a DVE chain)."""
                fth, hh = h // 2, h % 2
                prow = slice(hh * 64, hh * 64 + 64)
                tbg = t * NQC + qc
                tp = ps512.tile([128, 128], bf16, tag="ps512")
                nc.tensor.transpose(
                    tp[0:64, :], attn_sb[:, tbg, h * 64 : (h + 1) * 64], ident
                )
                nc.vector.tensor_copy(
                    out=attnT_sb[prow, fth, tbg * 128 : (tbg + 1) * 128],
                    in_=tp[0:64, :],
                )

            Copy = mybir.ActivationFunctionType.Copy

            def emit_out_proj(tb):
                ot = outs.tile([128, 1024], bf16, tag="outstage")
                for ob in range(2):
                    ps = ps512.tile([128, 512], f32, tag="ps512")
                    for ftt in range(2):
                        nc.tensor.matmul(
                            ps,
                            attnT_sb[:, ftt, tb * 128 : (tb + 1) * 128],
                            wo_sb[:, ftt, ob * 512 : (ob + 1) * 512],
                            start=(ftt == 0),
                            stop=(ftt == 1),
                        )
                    if ob == 0:
                        nc.scalar.activation(
                            out=ot[:, 0:512], in_=ps, func=Copy,
                        )
                    else:
                        nc.vector.tensor_copy(out=ot[:, 512:1024], in_=ps)
                nc.sync.dma_start(
                    out=out_d[tb * 128 : (tb + 1) * 128, :], in_=ot
                )

            # ---------------- schedule ----------------
            # Span ~= DMA lead-in + total PE busy + drain, so the only goals
            # are: start PE as soon as the first DMA chunks land, never let a
            # PE instruction reach the (in-order) queue head before its
            # producers finished, and keep the drain short.  Cross-engine
            # consumers are therefore lagged behind their producers.

            # P0: earliest PE work in DMA-arrival order
            # (kap, xt[0:128], wq, wk, xt[128:512], xt1, xt2, xt3, wv, wo, bm)
            emit_q_proj_cols(0, 0, 128)
            emit_k_proj(0, 0)
            emit_q_proj_cols(0, 128, 512)
            for ktg in range(1, 4):
                emit_k_proj(0, ktg)
            emit_q_proj(0, 1)
            emit_k_proj_cols(0, 512, 1024)
            emit_q_proj(0, 2)

            # P1: heads 0/1 scores (track 0 then track 1) with the remaining
            # projections woven in as PE filler.
            fillers = []
            fillers += [lambda: emit_k_proj_cols(0, 1024, 1536)]
            fillers += [lambda: emit_q_proj(0, 3)]
            fillers += [lambda q=q: emit_q_proj(1, q) for q in range(2)]
            fillers += [lambda: emit_k_proj_cols(0, 1536, 2048)]
            fillers += [lambda q=q: emit_q_proj(1, q) for q in range(2, 4)]
            fillers += [lambda b=b: emit_k_proj_cols(1, b * 512, (b + 1) * 512)
                        for b in range(4)]
            fil = iter(fillers)

            def pop_fillers(n):
                for _ in range(n):
                    f = next(fil, None)
                    if f is not None:
                        f()

            for i in range(NQC):
                emit_score_tile(0, 0, i)
                emit_score_tile(1, 0, i)
                pop_fillers(3)
            for i in range(NQC):
                pop_fillers(2)
                emit_score_tile(0, 1, i)
                emit_score_tile(1, 1, i)
            pop_fillers(len(fillers))

            # P2: heads 2/3 scores + heads 0/1 AV + v projection, all lagged.
            # Chunk list alternates tracks by ascending q-chunk so each
            # chunk's v tiles (j and 8+j) arrive just before it is needed.
            tiles = [(t, i) for t in range(2) for i in range(NQC)]
            chunks = [(t, qc) for qc in range(NQC) for t in range(2)]

            def av_pair_a(c):
                emit_av_a(0, c[0], c[1])
                emit_av_a(1, c[0], c[1])

            def av_pair_b(c):
                emit_av_b(0, c[0], c[1])
                emit_av_b(1, c[0], c[1])

            for s in range(16):
                t, i = tiles[s]
                emit_score_tile(2, t, i)
                emit_score_tile(3, t, i)
                if s < 8:
                    emit_v_proj(s)
                    emit_v_proj(8 + s)
                if s >= 2:
                    av_pair_a(chunks[s - 2])
                if s >= 3:
                    av_pair_b(chunks[s - 3])
            for s in range(16, 19):
                if s - 2 < 16:
                    av_pair_a(chunks[s - 2])
                if s - 3 < 16:
                    av_pair_b(chunks[s - 3])

            # P3: heads 2/3 AV with the output projection, lagged likewise.
            def tb_of(c):
                return c[0] * NQC + c[1]

            for s in range(18):
                if s < 16:
                    emit_av_a(2, *chunks[s])
                    emit_av_a(3, *chunks[s])
                if 0 <= s - 1 < 16:
                    emit_av_b(2, *chunks[s - 1])
                    emit_av_b(3, *chunks[s - 1])
                if s >= 2:
                    emit_out_proj(tb_of(chunks[s - 2]))
    nc.finalize()
    return nc


def _build_generic():
    nc = bacc.Bacc()
    f32, bf16 = DT.float32, DT.bfloat16

    xT = nc.declare_dram_parameter("xT", [128, 8, S], bf16, isOutput=False)
    wq = nc.declare_dram_parameter("wq", [128, 8, FPC], bf16, isOutput=False)
    wk = nc.declare_dram_parameter("wk", [128, 8, FPC], bf16, isOutput=False)
    wv = nc.declare_dram_parameter("wv", [128, 8, FPC], bf16, isOutput=False)
    wo = nc.declare_dram_parameter("wo", [128, 2, D], bf16, isOutput=False)
    bq = nc.declare_dram_parameter("bq", [128, 2], f32, isOutput=False)
    bk = nc.declare_dram_parameter("bk", [128, 2], f32, isOutput=False)
    maskT = nc.declare_dram_parameter("maskT", [S, S], bf16, isOutput=False)
    out_d = nc.declare_dram_parameter("out", [S, D], f32, isOutput=True)

    with tile.TileContext(nc) as tc:
        with (
            tc.tile_pool(name="consts", bufs=1) as consts,
            tc.tile_pool(name="pp", bufs=36) as ppool,
            tc.tile_pool(name="small", bufs=6) as small,
            tc.tile_pool(name="mp", bufs=4) as mpool,
            tc.tile_pool(name="ps512", bufs=2, space="PSUM") as ps512,
            tc.tile_pool(name="scps", bufs=2, space="PSUM") as scps,
            tc.tile_pool(name="avps", bufs=2, space="PSUM") as avps,
        ):
            Exp = mybir.ActivationFunctionType.Exp

            xt_sb = consts.tile([128, 8, S], bf16)
            nc.sync.dma_start(out=xt_sb, in_=xT[:, :, :])
            wq_sb = consts.tile([128, 8, FPC], bf16)
            nc.sync.dma_start(out=wq_sb, in_=wq[:, :, :])
            wk_sb = consts.tile([128, 8, FPC], bf16)
            nc.sync.dma_start(out=wk_sb, in_=wk[:, :, :])
            wv_sb = consts.tile([128, 8, FPC], bf16)
            nc.sync.dma_start(out=wv_sb, in_=wv[:, :, :])
            wo_sb = consts.tile([128, 2, D], bf16)
            nc.sync.dma_start(out=wo_sb, in_=wo[:, :, :])
            bq_sb = consts.tile([128, 2], f32)
            nc.sync.dma_start(out=bq_sb, in_=bq[:, :])
            bk_sb = consts.tile([128, 2], f32)
            nc.sync.dma_start(out=bk_sb, in_=bk[:, :])

            m20 = consts.tile([128, 1], f32)
            nc.vector.memset(m20, -20.0)
            ident = consts.tile([128, 128], bf16)
            make_identity(nc, ident)

            qT_sb = consts.tile([128, 2, S], bf16)
            kT_sb = consts.tile([128, 2, S], bf16)
            v_sb = consts.tile([128, 16, HPC * (HD + 1)], bf16)
            v4 = v_sb.rearrange("p k (h c) -> p k h c", c=HD + 1)
            nc.gpsimd.memset(v4[:, :, :, HD : HD + 1], 1.0)
            attn_sb = consts.tile([128, 16, FPC], bf16)
            attnT_sb = consts.tile([128, 2, S], bf16)


            for dst, w_sb, b_sb in ((qT_sb, wq_sb, bq_sb), (kT_sb, wk_sb, bk_sb)):
                for ft in range(2):
                    for qb in range(4):
                        ps = ps512.tile([128, 512], f32, tag="ps512")
                        for dt_i in range(8):
                            nc.tensor.matmul(
                                ps,
                                w_sb[:, dt_i, ft * 128 : (ft + 1) * 128],
                                xt_sb[:, dt_i, qb * 512 : (qb + 1) * 512],
                                start=(dt_i == 0),
                                stop=(dt_i == 7),
                            )
                        nc.vector.tensor_scalar_add(
                            out=dst[:, ft, qb * 512 : (qb + 1) * 512],
                            in0=ps,
                            scalar1=b_sb[:, ft : ft + 1],
                        )
            for tb in range(16):
                ps = ps512.tile([128, FPC], f32, tag="ps512")
                for dt_i in range(8):
                    nc.tensor.matmul(
                        ps,
                        xt_sb[:, dt_i, tb * 128 : (tb + 1) * 128],
                        wv_sb[:, dt_i, :],
                        start=(dt_i == 0),
                        stop=(dt_i == 7),
                    )
                nc.any.tensor_copy(
                    out=v4[:, tb, :, 0:HD],
                    in_=ps.rearrange("p (h c) -> p h c", c=HD),
                )

            NQC = TL // 128
            for h in range(HPC):
                fth, hh = h // 2, h % 2
                prow = slice(hh * 64, hh * 64 + 64)

                def _av_block(tbg, mms, h=h, fth=fth, prow=prow):
                    av = avps.tile([128, HD + 1], f32, tag="av")
                    for j, (lh, ktg) in enumerate(mms):
                        nc.tensor.matmul(
                            av, lh, v4[:, ktg, h, :],
                            start=(j == 0), stop=(j == len(mms) - 1),
                        )
                    r = small.tile([128, 1], f32, tag="recip")
                    nc.vector.reciprocal(r, av[:, HD : HD + 1])
                    nc.vector.tensor_scalar_mul(
                        attn_sb[:, tbg, h * 64 : (h + 1) * 64], av[:, 0:HD], r
                    )
                    tp = avps.tile([128, 128], bf16, tag="av")
                    nc.tensor.transpose(
                        tp[0:64, :], attn_sb[:, tbg, h * 64 : (h + 1) * 64], ident
                    )
                    nc.vector.tensor_copy(
                        out=attnT_sb[prow, fth, tbg * 128 : (tbg + 1) * 128],
                        in_=tp[0:64, :],
                    )

                for half in range(2):
                    ptiles = {}
                    for ktg in range(16):
                        lhsT = kT_sb[prow, fth, ktg * 128 : (ktg + 1) * 128]
                        sc = scps.tile([128, 1024], f32, tag="scps")
                        for seg in range(2):
                            qg = half * 1024 + seg * 512
                            nc.tensor.matmul(
                                sc[:, seg * 512 : (seg + 1) * 512],
                                lhsT,
                                qT_sb[prow, fth, qg : qg + 512],
                                start=True,
                                stop=True,
                            )
                        pt = ppool.tile([128, 1024], bf16, tag="pp")
                        nc.scalar.activation(
                            out=pt[:, 0:1024], in_=sc[:, 0:1024], func=Exp,
                            bias=m20, scale=1.0,
                        )
                        mt = mpool.tile([128, 1024], bf16, tag="mp")
                        nc.sync.dma_start(
                            out=mt,
                            in_=maskT[ktg * 128 : (ktg + 1) * 128,
                                      half * 1024 : (half + 1) * 1024],
                        )
                        nc.vector.tensor_mul(pt[:, 0:1024], pt[:, 0:1024], mt)
                        ptiles[ktg] = pt
                    for qc in range(NQC):
                        mms = [
                            (ptiles[ktg][:, 128 * qc : 128 * qc + 128], ktg)
                            for ktg in range(16)
                        ]
                        _av_block(half * NQC + qc, mms)

            for tb in range(16):
                for ob in range(2):
                    ps = ps512.tile([128, 512], f32, tag="ps512")
                    for ftt in range(2):
                        nc.tensor.matmul(
                            ps,
                            attnT_sb[:, ftt, tb * 128 : (tb + 1) * 128],
                            wo_sb[:, ftt, ob * 512 : (ob + 1) * 512],
                            start=(ftt == 0),
                            stop=(ftt == 1),
                        )
                    ot = small.tile([128, 512], f32, tag="outstage")
                    nc.any.tensor_copy(out=ot, in_=ps)
                    nc.sync.dma_start(
                        out=out_d[tb * 128 : (tb + 1) * 128, ob * 512 : (ob + 1) * 512],
                        in_=ot,
                    )
    nc.finalize()
    return nc


def _get_nc(structured: bool):
    key = "structured" if structured else "generic"
    if key not in _cache:
        _cache[key] = _build_structured() if structured else _build_generic()
    return _cache[key]


def kernel(x, cross_track_mask, w_qkv, b_qkv, w_out, b_out):
    x = np.asarray(x, dtype=np.float32)
    mask = np.asarray(cross_track_mask).astype(bool)
    w_qkv = np.asarray(w_qkv, dtype=np.float32)
    b_qkv = np.asarray(b_qkv, dtype=np.float32)
    w_out = np.asarray(w_out, dtype=np.float32)
    b_out = np.asarray(b_out, dtype=np.float32)

    structured = bool(np.array_equal(mask, np.broadcast_to(_structured_mask(), mask.shape)))
    nc = _get_nc(structured)

    scale = 1.0 / np.sqrt(np.float32(HD))
    b_v = b_qkv[2 * D :]
    b_out_adj = (b_out + b_v @ w_out).astype(np.float32)

    in_maps = []
    for c in range(N_CORES):
        b = c // (N_CORES // B)
        g = c % (N_CORES // B)
        fs = slice(g * FPC, (g + 1) * FPC)

        xT_c = np.ascontiguousarray(
            x[b].T.reshape(8, 128, S).transpose(1, 0, 2)
        ).astype(BF16)

        def wslice(off):
            w = w_qkv[:, off + g * FPC : off + (g + 1) * FPC]
            return np.ascontiguousarray(
                w.reshape(8, 128, FPC).transpose(1, 0, 2)
            )

        wq_c = (wslice(0) * scale).astype(BF16)
        wk_c = wslice(D).astype(BF16)
        wv_c = wslice(2 * D).astype(BF16)
        wo_c = np.ascontiguousarray(
            w_out[fs].reshape(2, 128, D).transpose(1, 0, 2)
        ).astype(BF16)

        m = {
            "xT": xT_c,
            "wq": wq_c,
            "wk": wk_c,
            "wv": wv_c,
            "wo": wo_c,
        }
        if structured:
            # kappa(j) = b_q,h . k_h(j) (scaled): the per-key score offset
            # from the query bias, folded into the exp bias on device.
            # Computed host-side (a [S,D]@[D,HPC] matvec, ~0.03% of flops).
            wk_full = w_qkv[:, D + g * FPC : D + (g + 1) * FPC]
            bq_full = b_qkv[g * FPC : (g + 1) * FPC] * scale
            wkb_c = np.stack(
                [
                    wk_full[:, h * HD : (h + 1) * HD]
                    @ bq_full[h * HD : (h + 1) * HD]
                    for h in range(HPC)
                ],
                axis=1,
            )  # [1024, HPC]
            kap_c = (
                x[b].astype(BF16).astype(np.float32)
                @ wkb_c.astype(BF16).astype(np.float32)
            ) - 20.0  # [S, HPC]
            m["kap"] = np.ascontiguousarray(
                kap_c.reshape(16, 128, HPC).transpose(1, 0, 2)
            ).astype(np.float32)
            ar = np.arange(128)
            m["bm"] = ((ar[:, None] // BAR) == (ar[None, :] // BAR)).astype(BF16)
        else:
            m["bq"] = np.ascontiguousarray(
                (b_qkv[fs] * scale).reshape(2, 128).T
            ).astype(np.float32)
            m["bk"] = np.ascontiguousarray(
                b_qkv[D + g * FPC : D + (g + 1) * FPC].reshape(2, 128).T
            ).astype(np.float32)
            m["maskT"] = np.ascontiguousarray(mask[b].T).astype(BF16)
        in_maps.append(m)

    res = run_bass_kernel_spmd(nc, in_maps, list(range(N_CORES)))

    out = np.empty((B, S, D), dtype=np.float32)
    gpb = N_CORES // B
    for b in range(B):
        acc = res.results[b * gpb]["out"].astype(np.float32)
        for g in range(1, gpb):
            acc = acc + res.results[b * gpb + g]["out"].astype(np.float32)
        out[b] = acc + b_out_adj
    return out
